# revision 9
# baseline (speedup 1.0000x reference)
"""Depth-aware 3x3 convolution on 8 Trainium2 NeuronCores (Bass, raw engine blocks).

out[b,o,h,w] = sum_{c,kh,kw} weight[o,c,kh,kw] * x[b,c,h+kh-1,w+kw-1]
                             * exp(-8.3*|depth[b,h,w] - depth[b,h+kh-1,w+kw-1]|)

Sharding: core = 2*b + (h >= 128); each core computes a [32, 128, 256] output
slab from a 130-row padded input frame (1-row halo from the host slice).

v4 pipeline (fp16 on-chip, 8 groups of 16 rows = 4096 px):
  A. sim phase: |dc-dk| via scalar_tensor_tensor (4x DVE mode) -> exp (ACT)
     -> 3 DMA stores to DRAM simd[9, 32768].
  B. x3 loaded straight from DRAM 3x with -1/0/+1 flat column shifts into the
     three 32-partition j-blocks (no SBUF->SBUF shift copies).
  C. loop over 8 groups g (double-buffered):
     - bcasts: per (g, j) ONE fused DMA (center tap slot pre-memset to 1.0):
       simd rows {j, 3+j, 6+j} px-slice -> simrep3[32j:32j+32, 3 t-slots],
       src AP [[0,32],[3*NPIX,3],[1,GPIX]]. j=0/2 on sync queue, j=1 on
       scalar queue.
     - DVE: per t one scalar_tensor_tensor (x*1.0)*sim [96, 16, 256] (4x mode)
     - PE : per t 8 matmuls K=96 N=512; psum out at partition base 32*s
       (tile_position cols) so psum holds [128 = 4 px-chunks x 32 oc, 1024].
     - ACT: one [128, 1024] psum -> out_sb fp16 copy per group.
     - sync: one [128, 1024] out store per group.
"""
import sys

import numpy as np

sys.path.insert(0, "/opt/trn_rl_repo")

import concourse.bass as bass
import concourse.mybir as mybir
from concourse.bass_utils import run_bass_kernel_spmd

F32 = mybir.dt.float32
F16 = mybir.dt.float16
EXP = mybir.ActivationFunctionType.Exp
MULT = mybir.AluOpType.mult
ADD = mybir.AluOpType.add
MAX = mybir.AluOpType.max

B, C, H, W = 4, 32, 256, 256
O = 32
ALPHA = 8.3
R = 128  # output rows per core
WP = W + 2  # padded width
FR = R + 2  # frame rows per core
FLAT = FR * WP  # 33540
NPIX = R * W  # 32768
GROWS = 16  # rows per group
GPIX = GROWS * W  # 4096
NG = R // GROWS  # 8 groups
SUB = 1024  # px per psum partition-chunk (s)
MMN = 512  # matmul free-dim chunk (PSUM bank limit)
H0 = 66 * WP  # x first-half split (row 66) covers groups 0-3 incl halo


def build_nc():
    nc = bass.Bass("TRN2", target_bir_lowering=False, debug=False, num_devices=8)
    x_in = nc.declare_dram_parameter("x", [C, FLAT], F16, isOutput=False)
    dp_in = nc.declare_dram_parameter("dp", [FR, WP], F16, isOutput=False)
    w3_in = nc.declare_dram_parameter("w3", [96, 96], F16, isOutput=False)
    out_d = nc.declare_dram_parameter("out", [O, NPIX], F16, isOutput=True)
    simd = nc.dram_tensor("simd", [9, NPIX], F16)

    from contextlib import ExitStack

    ctx = ExitStack()
    with ctx:
        d_sb = ctx.enter_context(nc.sbuf_tensor([128, 3 * WP], F16))
        adiff9 = ctx.enter_context(nc.sbuf_tensor([128, 9 * W], F16))
        sim9 = ctx.enter_context(nc.sbuf_tensor([128, 9 * W], F16))
        w3_sb = ctx.enter_context(nc.sbuf_tensor([96, 96], F16))
        x3c = ctx.enter_context(nc.sbuf_tensor([96, FLAT], F16))
        simrep3 = ctx.enter_context(nc.sbuf_tensor([96, 2 * 3 * GPIX], F16))
        xm3 = ctx.enter_context(nc.sbuf_tensor([96, 2 * 3 * GPIX], F16))
        out_sb = ctx.enter_context(nc.sbuf_tensor([128, 2 * SUB], F16))
        psum = ctx.enter_context(nc.psum_tensor([128, 2 * SUB], F32))
        ld_d = ctx.enter_context(nc.semaphore("ld_d"))
        ld_d2 = ctx.enter_context(nc.semaphore("ld_d2"))
        w_sem = ctx.enter_context(nc.semaphore("w_sem"))
        x_sem = ctx.enter_context(nc.semaphore("x_sem"))
        xh1_sem = ctx.enter_context(nc.semaphore("xh1_sem"))
        sim_dve = ctx.enter_context(nc.semaphore("sim_dve"))
        act_exp = ctx.enter_context(nc.semaphore("act_exp"))
        sst = ctx.enter_context(nc.semaphore("sst"))
        bs = [ctx.enter_context(nc.semaphore(f"bs{p}")) for p in range(2)]
        mod_sem = ctx.enter_context(nc.semaphore("mod_sem"))
        pe_g = ctx.enter_context(nc.semaphore("pe_g"))
        cp_sem = ctx.enter_context(nc.semaphore("cp_sem"))
        st_sem = ctx.enter_context(nc.semaphore("st_sem"))
        ones_sem = ctx.enter_context(nc.semaphore("ones_sem"))
        block = ctx.enter_context(nc.Block())

        # x3c row view [p, frame_row, col]
        x3c_r = x3c.ap().rearrange("p (r w) -> p r w", w=WP)
        # xm3 / simrep3 group-tap views: [p, buf, t, r, w]
        xm3_v = xm3.ap().rearrange("p (b t r w) -> p b t r w", b=2, t=3, w=W)
        sr3_v = simrep3.ap().rearrange("p (b t r w) -> p b t r w", b=2, t=3, w=W)
        # simd tap view [j, t, px]
        simd_jt = simd.ap().rearrange("(t j) px -> j t px", j=3)
        # out DRAM view [g, s, o, px]: partition (s, o) of group g -> px chunk
        out_gsop = out_d.ap().rearrange("o (g s px) -> g s o px", g=NG, s=4)

        def bcast_dma(eng, g, j):
            # one DMA: simd rows {j, 3+j, 6+j} (t slots; j==1 skips the center
            # t=1 slot, pre-filled with ones) -> simrep3 j-block, buf g%2
            base = (g % 2) * 3 * GPIX
            if j == 1:
                src = simd_jt[1][0::2, g * GPIX : (g + 1) * GPIX]
                dst = simrep3.ap()[32:64].rearrange("p (t px) -> p t px", t=6)[
                    :, (g % 2) * 3 : (g % 2) * 3 + 3 : 2
                ]
            else:
                src = simd_jt[j][:, g * GPIX : (g + 1) * GPIX]
                dst = simrep3[32 * j : 32 * (j + 1), base : base + 3 * GPIX]
            return eng.dma_start(dst, src.partition_broadcast(32))

        @block.sync
        def _(sync: bass.BassEngine):
            # d row-shifted views (t=0/1 gate the first six subs), weights
            for t in range(3):
                sync.dma_start(
                    d_sb[:, t * WP : (t + 1) * WP], dp_in[t : t + 128, :]
                ).then_inc(ld_d2 if t == 2 else ld_d, 16)
            sync.dma_start(w3_sb[:], w3_in[:]).then_inc(w_sem, 16)

            # x: three flat column-shifted copies straight from DRAM, split in
            # halves at row 66 so groups 0-3 unblock early. Unwritten edge
            # elements (col 0 of j=0, col 257 tail of j=2) are never read.
            def xload(j, lo, hi):
                if j == 0:
                    d0, s0, n = lo + 1, lo, min(hi + 1, FLAT) - (lo + 1)
                elif j == 1:
                    d0, s0, n = lo, lo, hi - lo
                else:
                    d0, s0, n = lo, lo + 1, min(hi + 1, FLAT) - (lo + 1)
                sync.dma_start(
                    x3c[32 * j : 32 * (j + 1), d0 : d0 + n],
                    x_in[:, s0 : s0 + n],
                ).then_inc(x_sem if lo == 0 else xh1_sem, 16)

            for j in range(3):
                xload(j, 0, H0)

            def simstore(t):
                sync.wait_ge(act_exp, t + 1)
                sync.dma_start(
                    simd.ap()[3 * t : 3 * t + 3].rearrange("k (r w) -> r k w", w=W),
                    sim9.ap()[:, 3 * t * W : (3 * t + 3) * W].rearrange(
                        "p (k w) -> p k w", w=W
                    ),
                ).then_inc(sst, 16)

            # sim stores pipelined per tap-row; x second halves fill the gaps
            simstore(0)
            xload(0, H0, FLAT)
            simstore(1)
            xload(1, H0, FLAT)
            simstore(2)
            xload(2, H0, FLAT)

            # head broadcasts for groups 0/1 (j = 0, 2)
            sync.wait_ge(sst, 48)
            for g in (0, 1):
                for j in (0, 2):
                    bcast_dma(sync, g, j).then_inc(bs[g % 2], 16)
            for g in range(NG):
                if g + 2 < NG:
                    sync.wait_ge(mod_sem, 3 * g + 3)
                    for j in (0, 2):
                        bcast_dma(sync, g + 2, j).then_inc(bs[g % 2], 16)
                sync.wait_ge(cp_sem, g + 1)
                sync.dma_start(
                    out_gsop[g],
                    out_sb[:, (g % 2) * SUB : (g % 2 + 1) * SUB],
                ).then_inc(st_sem, 16)

        @block.gpsimd
        def _(pool):
            # center-tap (t=1, j=1) simrep3 slots pre-filled with 1.0
            for b in range(2):
                pool.memset(
                    simrep3[32:64, b * 3 * GPIX + GPIX : b * 3 * GPIX + 2 * GPIX],
                    1.0,
                ).then_inc(ones_sem, 1)

        @block.vector
        def _(vector):
            # sim phase: diff via STT (4x mode), then abs via STT, per t-row
            vector.wait_ge(ld_d, 32)
            for k in range(9):
                if k == 6:
                    vector.wait_ge(ld_d2, 16)
                vector.scalar_tensor_tensor(
                    adiff9[:, k * W : (k + 1) * W],
                    d_sb[:, (k // 3) * WP + k % 3 : (k // 3) * WP + k % 3 + W],
                    -1.0,
                    d_sb[:, WP + 1 : WP + 1 + W],
                    op0=MULT,
                    op1=ADD,
                )
                if k % 3 == 2:
                    t = k // 3
                    vector.scalar_tensor_tensor(
                        adiff9[:, 3 * t * W : (3 * t + 3) * W],
                        adiff9[:, 3 * t * W : (3 * t + 3) * W],
                        -1.0,
                        adiff9[:, 3 * t * W : (3 * t + 3) * W],
                        op0=MULT,
                        op1=MAX,
                    ).then_inc(sim_dve, 3)
            # modulation loop: per (g, t) one (x * 1.0) * sim  [96, 16, 256]
            for g in range(NG):
                bi = g % 2
                for t in range(3):
                    if t == 0:
                        vector.wait_ge(x_sem, 48)
                        if g >= 4:
                            vector.wait_ge(xh1_sem, 48)
                        vector.wait_ge(bs[bi], 48 * (g // 2 + 1))
                        if g >= 2:
                            vector.wait_ge(pe_g, g - 1)
                    if g < 2 and t == 1:
                        vector.wait_ge(ones_sem, 2)
                    vector.scalar_tensor_tensor(
                        xm3_v[:, bi, t],
                        x3c_r[:, 16 * g + t : 16 * g + t + GROWS, 1 : 1 + W],
                        1.0,
                        sr3_v[:, bi, t],
                        op0=MULT,
                        op1=MULT,
                    ).then_inc(mod_sem, 1)

        @block.tensor
        def _(tensor):
            tensor.wait_ge(w_sem, 16)
            for g in range(NG):
                for t in range(3):
                    tensor.wait_ge(mod_sem, 3 * g + t + 1)
                    if t == 0 and g >= 2:
                        tensor.wait_ge(cp_sem, g - 1)
                    base = (g % 2) * 3 * GPIX + t * GPIX
                    for s in range(4):
                        for qq in range(2):
                            mm = tensor.matmul(
                                psum[
                                    32 * s : 32 * (s + 1),
                                    (g % 2) * SUB
                                    + qq * MMN : (g % 2) * SUB
                                    + (qq + 1) * MMN,
                                ],
                                w3_sb[:, 32 * t : 32 * (t + 1)],
                                xm3[
                                    :,
                                    base
                                    + s * SUB
                                    + qq * MMN : base
                                    + s * SUB
                                    + (qq + 1) * MMN,
                                ],
                                start=(t == 0),
                                stop=(t == 2),
                                tile_position=(0, 32 * s),
                            )
                    if t == 2:
                        mm.then_inc(pe_g, 1)

        @block.scalar
        def _(scalar):
            # exp per tap-row t
            for t in range(3):
                scalar.wait_ge(sim_dve, 3 * (t + 1))
                scalar.activation(
                    sim9[:, 3 * t * W : (3 * t + 3) * W],
                    adiff9[:, 3 * t * W : (3 * t + 3) * W],
                    EXP,
                    scale=-ALPHA,
                ).then_inc(act_exp, 1)
            # head j=1 broadcasts for groups 0/1
            scalar.wait_ge(sst, 48)
            for g in (0, 1):
                bcast_dma(scalar, g, 1).then_inc(bs[g % 2], 16)
            # steady state: j=1 broadcast for g+2, then [128, 1024] psum copy
            for g in range(NG):
                if g + 2 < NG:
                    scalar.wait_ge(mod_sem, 3 * g + 3)
                    bcast_dma(scalar, g + 2, 1).then_inc(bs[g % 2], 16)
                scalar.wait_ge(pe_g, g + 1)
                if g >= 2:
                    scalar.wait_ge(st_sem, 16 * (g - 1))
                scalar.copy(
                    out_sb[:, (g % 2) * SUB : (g % 2 + 1) * SUB],
                    psum[:, (g % 2) * SUB : (g % 2 + 1) * SUB],
                ).then_inc(cp_sem, 1)

    return nc


_NC_CACHE = None


def _get_nc():
    global _NC_CACHE
    if _NC_CACHE is None:
        _NC_CACHE = build_nc()
    return _NC_CACHE


def _prep_core(x, depth, core):
    b, half = core // 2, core % 2
    r0 = half * R
    # padded frame [C, FR, WP]: image rows r0-1 .. r0+R, zero-padded
    xpad = np.zeros((C, FR, WP), dtype=np.float16)
    dpad = np.zeros((FR, WP), dtype=np.float16)
    lo, hi = r0 - 1, r0 + R + 1
    slo, shi = max(lo, 0), min(hi, H)
    xpad[:, slo - lo : shi - lo, 1 : 1 + W] = x[b, :, slo:shi, :]
    dpad[slo - lo : shi - lo, 1 : 1 + W] = depth[b, 0, slo:shi, :]
    return {
        "x": xpad.reshape(C, FLAT),
        "dp": dpad,
    }


def make_in_maps(x, depth, weight):
    x = np.ascontiguousarray(x, dtype=np.float32)
    depth = np.ascontiguousarray(depth, dtype=np.float32)
    weight = np.ascontiguousarray(weight, dtype=np.float32)
    # w3[32j + c, 32t + o] = weight[o, c, t, j]
    w3 = np.transpose(weight, (3, 1, 2, 0)).reshape(96, 96).astype(np.float16)
    in_maps = []
    for core in range(8):
        m = _prep_core(x, depth, core)
        m["w3"] = w3
        in_maps.append(m)
    return in_maps


def kernel(x, depth, weight):
    in_maps = make_in_maps(x, depth, weight)
    nc = _get_nc()
    res = run_bass_kernel_spmd(nc, in_maps, list(range(8)))

    out = np.empty((B, O, H, W), dtype=np.float32)
    for core in range(8):
        b, half = core // 2, core % 2
        out[b, :, half * R : (half + 1) * R, :] = (
            np.asarray(res.results[core]["out"]).astype(np.float32).reshape(O, R, W)
        )
    return out


# revision 10
# speedup vs baseline: 1.2003x; 1.2003x over previous
"""Depth-aware 3x3 convolution on 8 Trainium2 NeuronCores (Bass, raw engine blocks).

out[b,o,h,w] = sum_{c,kh,kw} weight[o,c,kh,kw] * x[b,c,h+kh-1,w+kw-1]
                             * exp(-8.3*|depth[b,h,w] - depth[b,h+kh-1,w+kw-1]|)

Sharding: core = 2*b + (h >= 128); each core computes a [32, 128, 256] output
slab from a 130-row padded input frame (1-row halo from the host slice).

v5 pipeline (fp16 on-chip, 8 groups of 16 rows = 4096 px). The kernel is
DMA-bandwidth-bound (~26 MB/core, ~200 GB/s aggregate), so DMAs are spread
over three queues (qSP / qAct / qPool) and simrep is triple-buffered for
3-group broadcast lookahead:
  A. sim phase: subs (DVE 2x) -> |.| (STT) -> exp (ACT) -> DRAM simd[9, NPIX].
  B. x3 loaded straight from DRAM 3x with -1/0/+1 flat column shifts into the
     three 32-partition j-blocks.
  C. loop over groups g:
     - bcasts: per (g, j) ONE fused DMA (center tap slot pre-memset to 1.0):
       simd rows {j, 3+j, 6+j} px-slice -> simrep3[32j:32j+32, 3 t-slots].
       j=0 on sync, j=2 on scalar, j=1 on gpsimd queue.
     - DVE: per t one tensor_mul (2x mode) [96, 16, 256]
     - PE : per t 8 matmuls K=96 N=512; psum out at partition base 32*s
       (tile_position) so psum holds [128 = 4 px-chunks x 32 oc, 1024].
     - ACT: one [128, 1024] psum -> out_sb fp16 copy per group.
     - gpsimd: one [128, 1024] out store per group.
"""
import sys

import numpy as np

sys.path.insert(0, "/opt/trn_rl_repo")

import concourse.bass as bass
import concourse.mybir as mybir
from concourse.bass_utils import run_bass_kernel_spmd

F32 = mybir.dt.float32
F16 = mybir.dt.float16
EXP = mybir.ActivationFunctionType.Exp
MULT = mybir.AluOpType.mult
MAX = mybir.AluOpType.max

B, C, H, W = 4, 32, 256, 256
O = 32
ALPHA = 8.3
R = 128  # output rows per core
WP = W + 2  # padded width
FR = R + 2  # frame rows per core
FLAT = FR * WP  # 33540
NPIX = R * W  # 32768
GROWS = 16  # rows per group
GPIX = GROWS * W  # 4096
NG = R // GROWS  # 8 groups
NB = 3  # simrep buffers (broadcast lookahead)
SUB = 1024  # px per psum partition-chunk (s)
MMN = 512  # matmul free-dim chunk (PSUM bank limit)
H0 = 66 * WP  # x first-half split (row 66) covers groups 0-3 incl halo


def build_nc():
    nc = bass.Bass("TRN2", target_bir_lowering=False, debug=False, num_devices=8)
    x_in = nc.declare_dram_parameter("x", [C, FLAT], F16, isOutput=False)
    dp_in = nc.declare_dram_parameter("dp", [FR, WP], F16, isOutput=False)
    w3_in = nc.declare_dram_parameter("w3", [96, 96], F16, isOutput=False)
    out_d = nc.declare_dram_parameter("out", [O, NPIX], F16, isOutput=True)
    simd = nc.dram_tensor("simd", [9, NPIX], F16)

    from contextlib import ExitStack

    ctx = ExitStack()
    with ctx:
        d_sb = ctx.enter_context(nc.sbuf_tensor([128, 3 * WP], F16))
        adiff9 = ctx.enter_context(nc.sbuf_tensor([128, 9 * W], F16))
        sim9 = ctx.enter_context(nc.sbuf_tensor([128, 9 * W], F16))
        w3_sb = ctx.enter_context(nc.sbuf_tensor([96, 96], F16))
        x3c = ctx.enter_context(nc.sbuf_tensor([96, FLAT], F16))
        simrep3 = ctx.enter_context(nc.sbuf_tensor([96, NB * 3 * GPIX], F16))
        xm3 = ctx.enter_context(nc.sbuf_tensor([96, 2 * 3 * GPIX], F16))
        out_sb = ctx.enter_context(nc.sbuf_tensor([128, 2 * SUB], F16))
        psum = ctx.enter_context(nc.psum_tensor([128, 2 * SUB], F32))
        ld_d = ctx.enter_context(nc.semaphore("ld_d"))
        ld_d2 = ctx.enter_context(nc.semaphore("ld_d2"))
        w_sem = ctx.enter_context(nc.semaphore("w_sem"))
        x_sem = ctx.enter_context(nc.semaphore("x_sem"))
        xh1_sem = ctx.enter_context(nc.semaphore("xh1_sem"))
        sim_dve = ctx.enter_context(nc.semaphore("sim_dve"))
        act_exp = ctx.enter_context(nc.semaphore("act_exp"))
        sst = ctx.enter_context(nc.semaphore("sst"))
        bs = [ctx.enter_context(nc.semaphore(f"bs{p}")) for p in range(NB)]
        mod_sem = ctx.enter_context(nc.semaphore("mod_sem"))
        pe_g = ctx.enter_context(nc.semaphore("pe_g"))
        cp_sem = ctx.enter_context(nc.semaphore("cp_sem"))
        st_sem = ctx.enter_context(nc.semaphore("st_sem"))
        ones_sem = ctx.enter_context(nc.semaphore("ones_sem"))
        block = ctx.enter_context(nc.Block())

        # x3c row view [p, frame_row, col]
        x3c_r = x3c.ap().rearrange("p (r w) -> p r w", w=WP)
        # xm3 / simrep3 group-tap views: [p, buf, t, r, w]
        xm3_v = xm3.ap().rearrange("p (b t r w) -> p b t r w", b=2, t=3, w=W)
        sr3_v = simrep3.ap().rearrange("p (b t r w) -> p b t r w", b=NB, t=3, w=W)
        # simd tap view [j, t, px]
        simd_jt = simd.ap().rearrange("(t j) px -> j t px", j=3)
        # out DRAM view [g, s, o, px]: partition (s, o) of group g -> px chunk
        out_gsop = out_d.ap().rearrange("o (g s px) -> g s o px", g=NG, s=4)

        def bcast_dma(eng, g, j):
            # one DMA: simd rows {j, 3+j, 6+j} (t slots; j==1 skips the center
            # t=1 slot, pre-filled with ones) -> simrep3 j-block, buf g%NB
            base = (g % NB) * 3 * GPIX
            if j == 1:
                src = simd_jt[1][0::2, g * GPIX : (g + 1) * GPIX]
                dst = simrep3.ap()[32:64].rearrange(
                    "p (t px) -> p t px", t=3 * NB
                )[:, (g % NB) * 3 : (g % NB) * 3 + 3 : 2]
            else:
                src = simd_jt[j][:, g * GPIX : (g + 1) * GPIX]
                dst = simrep3[32 * j : 32 * (j + 1), base : base + 3 * GPIX]
            return eng.dma_start(dst, src.partition_broadcast(32)).then_inc(
                bs[g % NB], 16
            )

        @block.sync
        def _(sync: bass.BassEngine):
            # d row-shifted views (t=0/1 gate the first six subs), weights
            for t in range(3):
                sync.dma_start(
                    d_sb[:, t * WP : (t + 1) * WP], dp_in[t : t + 128, :]
                ).then_inc(ld_d2 if t == 2 else ld_d, 16)
            sync.dma_start(w3_sb[:], w3_in[:]).then_inc(w_sem, 16)

            # x: three flat column-shifted copies straight from DRAM, split in
            # halves at row 66 so groups 0-3 unblock early. Unwritten edge
            # elements (col 0 of j=0, col 257 tail of j=2) are never read.
            def xload(j, lo, hi):
                if j == 0:
                    d0, s0, n = lo + 1, lo, min(hi + 1, FLAT) - (lo + 1)
                elif j == 1:
                    d0, s0, n = lo, lo, hi - lo
                else:
                    d0, s0, n = lo, lo + 1, min(hi + 1, FLAT) - (lo + 1)
                sync.dma_start(
                    x3c[32 * j : 32 * (j + 1), d0 : d0 + n],
                    x_in[:, s0 : s0 + n],
                ).then_inc(x_sem if lo == 0 else xh1_sem, 16)

            for j in range(3):
                xload(j, 0, H0)

            def simstore(t):
                sync.wait_ge(act_exp, t + 1)
                sync.dma_start(
                    simd.ap()[3 * t : 3 * t + 3].rearrange("k (r w) -> r k w", w=W),
                    sim9.ap()[:, 3 * t * W : (3 * t + 3) * W].rearrange(
                        "p (k w) -> p k w", w=W
                    ),
                ).then_inc(sst, 16)

            # sim stores pipelined per tap-row; x second halves fill the gaps
            simstore(0)
            xload(0, H0, FLAT)
            simstore(1)
            xload(1, H0, FLAT)
            simstore(2)
            xload(2, H0, FLAT)

            # j=0 broadcasts: head (g=0..NB-1), then lookahead g+NB
            sync.wait_ge(sst, 48)
            for g in range(NB):
                bcast_dma(sync, g, 0)
            for g in range(NG - NB):
                sync.wait_ge(mod_sem, 3 * g + 3)
                bcast_dma(sync, g + NB, 0)

        @block.gpsimd
        def _(pool):
            # center-tap (t=1, j=1) simrep3 slots pre-filled with 1.0
            for b in range(NB):
                pool.memset(
                    simrep3[32:64, b * 3 * GPIX + GPIX : b * 3 * GPIX + 2 * GPIX],
                    1.0,
                ).then_inc(ones_sem, 1)
            # j=1 broadcasts + out stores on the pool SWDGE queue
            pool.wait_ge(sst, 48)
            for g in range(NB):
                bcast_dma(pool, g, 1)
            for g in range(NG):
                if g + NB < NG:
                    pool.wait_ge(mod_sem, 3 * g + 3)
                    bcast_dma(pool, g + NB, 1)
                pool.wait_ge(cp_sem, g + 1)
                pool.dma_start(
                    out_gsop[g],
                    out_sb[:, (g % 2) * SUB : (g % 2 + 1) * SUB],
                ).then_inc(st_sem, 16)

        @block.vector
        def _(vector):
            # sim phase: diff (2x) + one drain + |.| via STT, per t-row
            vector.wait_ge(ld_d, 32)
            for k in range(9):
                if k == 6:
                    vector.wait_ge(ld_d2, 16)
                vector.tensor_sub(
                    adiff9[:, k * W : (k + 1) * W],
                    d_sb[:, WP + 1 : WP + 1 + W],
                    d_sb[:, (k // 3) * WP + k % 3 : (k // 3) * WP + k % 3 + W],
                )
            vector.drain()
            for t in range(3):
                vector.scalar_tensor_tensor(
                    adiff9[:, 3 * t * W : (3 * t + 3) * W],
                    adiff9[:, 3 * t * W : (3 * t + 3) * W],
                    -1.0,
                    adiff9[:, 3 * t * W : (3 * t + 3) * W],
                    op0=MULT,
                    op1=MAX,
                ).then_inc(sim_dve, 3)
            # modulation loop: per (g, t) one tensor_mul [96, 16, 256]
            for g in range(NG):
                bi = g % 2
                for t in range(3):
                    if t == 0:
                        vector.wait_ge(x_sem, 48)
                        if g >= 4:
                            vector.wait_ge(xh1_sem, 48)
                        vector.wait_ge(bs[g % NB], 48 * (g // NB + 1))
                        if g >= 2:
                            vector.wait_ge(pe_g, g - 1)
                    if g < NB and t == 1:
                        vector.wait_ge(ones_sem, NB)
                    vector.tensor_mul(
                        xm3_v[:, bi, t],
                        x3c_r[:, 16 * g + t : 16 * g + t + GROWS, 1 : 1 + W],
                        sr3_v[:, g % NB, t],
                    ).then_inc(mod_sem, 1)

        @block.tensor
        def _(tensor):
            tensor.wait_ge(w_sem, 16)
            for g in range(NG):
                for t in range(3):
                    tensor.wait_ge(mod_sem, 3 * g + t + 1)
                    if t == 0 and g >= 2:
                        tensor.wait_ge(cp_sem, g - 1)
                    base = (g % 2) * 3 * GPIX + t * GPIX
                    for s in range(4):
                        for qq in range(2):
                            mm = tensor.matmul(
                                psum[
                                    32 * s : 32 * (s + 1),
                                    (g % 2) * SUB
                                    + qq * MMN : (g % 2) * SUB
                                    + (qq + 1) * MMN,
                                ],
                                w3_sb[:, 32 * t : 32 * (t + 1)],
                                xm3[
                                    :,
                                    base
                                    + s * SUB
                                    + qq * MMN : base
                                    + s * SUB
                                    + (qq + 1) * MMN,
                                ],
                                start=(t == 0),
                                stop=(t == 2),
                                tile_position=(0, 32 * s),
                            )
                    if t == 2:
                        mm.then_inc(pe_g, 1)

        @block.scalar
        def _(scalar):
            # exp per tap-row t
            for t in range(3):
                scalar.wait_ge(sim_dve, 3 * (t + 1))
                scalar.activation(
                    sim9[:, 3 * t * W : (3 * t + 3) * W],
                    adiff9[:, 3 * t * W : (3 * t + 3) * W],
                    EXP,
                    scale=-ALPHA,
                ).then_inc(act_exp, 1)
            # j=2 broadcasts + [128, 1024] psum copies
            scalar.wait_ge(sst, 48)
            for g in range(NB):
                bcast_dma(scalar, g, 2)
            for g in range(NG):
                if g + NB < NG:
                    scalar.wait_ge(mod_sem, 3 * g + 3)
                    bcast_dma(scalar, g + NB, 2)
                scalar.wait_ge(pe_g, g + 1)
                if g >= 2:
                    scalar.wait_ge(st_sem, 16 * (g - 1))
                scalar.copy(
                    out_sb[:, (g % 2) * SUB : (g % 2 + 1) * SUB],
                    psum[:, (g % 2) * SUB : (g % 2 + 1) * SUB],
                ).then_inc(cp_sem, 1)

    return nc


_NC_CACHE = None


def _get_nc():
    global _NC_CACHE
    if _NC_CACHE is None:
        _NC_CACHE = build_nc()
    return _NC_CACHE


def _prep_core(x, depth, core):
    b, half = core // 2, core % 2
    r0 = half * R
    # padded frame [C, FR, WP]: image rows r0-1 .. r0+R, zero-padded
    xpad = np.zeros((C, FR, WP), dtype=np.float16)
    dpad = np.zeros((FR, WP), dtype=np.float16)
    lo, hi = r0 - 1, r0 + R + 1
    slo, shi = max(lo, 0), min(hi, H)
    xpad[:, slo - lo : shi - lo, 1 : 1 + W] = x[b, :, slo:shi, :]
    dpad[slo - lo : shi - lo, 1 : 1 + W] = depth[b, 0, slo:shi, :]
    return {
        "x": xpad.reshape(C, FLAT),
        "dp": dpad,
    }


def make_in_maps(x, depth, weight):
    x = np.ascontiguousarray(x, dtype=np.float32)
    depth = np.ascontiguousarray(depth, dtype=np.float32)
    weight = np.ascontiguousarray(weight, dtype=np.float32)
    # w3[32j + c, 32t + o] = weight[o, c, t, j]
    w3 = np.transpose(weight, (3, 1, 2, 0)).reshape(96, 96).astype(np.float16)
    in_maps = []
    for core in range(8):
        m = _prep_core(x, depth, core)
        m["w3"] = w3
        in_maps.append(m)
    return in_maps


def kernel(x, depth, weight):
    in_maps = make_in_maps(x, depth, weight)
    nc = _get_nc()
    res = run_bass_kernel_spmd(nc, in_maps, list(range(8)))

    out = np.empty((B, O, H, W), dtype=np.float32)
    for core in range(8):
        b, half = core // 2, core % 2
        out[b, :, half * R : (half + 1) * R, :] = (
            np.asarray(res.results[core]["out"]).astype(np.float32).reshape(O, R, W)
        )
    return out
